# revision 1
# baseline (speedup 1.0000x reference)
"""Trainium2 Bass kernel for nn_CTAttention (continuous-time sparse attention).

Shapes (hardcoded): B=8, L=1024, H=8, E=64, S=4.
Sharding: data-parallel over B (one batch element per NeuronCore, 8 cores),
head loop inside each core; the small E x E weights are replicated.

Math (per b, h), with tau = his_timeslot[b] (shared by q/k/v interp):
  Xq[f, l]   = sum_e Wq[f, e] x[l, e]          (projection commutes with the
                                                linear time-interp, so project
                                                first, interp after)
  ct_q[(s,f), l] = Xq[f, l] + tau[l, s] * (Xq[f, l+1] - Xq[f, l])   (clamped)
  scoresT[m, l]  = sum_{s,f} ct_k[(s,f), m] ct_q[(s,f), l]
  E = exp(0.0625 * scoresT) masked causally (no max-subtraction: logits are
      O(1) here so exp is safe in fp32)
  xi[m, :] = v[m] + (sum_s tau[m,s]/4) * (v[m+1] - v[m]);  v_bar = 2*Wv@xi
  OT[e', l] = sum_m xi_aug[m, e'] E[m, l]   (xi_aug has a ones column ->
                                             row 64 of OT = softmax denom)
  V[l, f] = (sum_e OT[e, l] * 2Wv^T[e, f]) / denom[l]
Biases bq/bk are zero in this problem (asserted); bv is handled exactly by
adding 2*bv to the output on the host (rows of softmax sum to 1).
"""

import numpy as np

B, L, H, E, S = 8, 1024, 8, 64, 4
P = 128           # partitions
NT = L // P       # 8 l-tiles of 128
NJ = L // 512     # 2 l-chunks of 512
EXP_SCALE = 0.5 / np.sqrt(E)  # 0.5 * SCALE = 0.5/8 = 0.0625

_CACHE = {}


def _build_program(ct_bf16: bool, dbg: bool = False):
    from contextlib import ExitStack

    import concourse.bass as bass
    import concourse.tile as tile
    from concourse import bacc, mybir

    f32 = mybir.dt.float32
    f32r = mybir.dt.float32r
    bf16 = mybir.dt.bfloat16
    op_dt = bf16 if ct_bf16 else f32r   # dtype of matmul operand tiles
    el_dt = bf16 if ct_bf16 else f32    # dtype of DVE-only intermediates
    Exp = mybir.ActivationFunctionType.Exp
    Alu = mybir.AluOpType

    nc = bacc.Bacc("TRN2", debug=False, enable_asserts=False, num_devices=8)

    qk_d = nc.dram_tensor("qk", [L, H, 2, E], f32, kind="ExternalInput").ap()
    v_d = nc.dram_tensor("v", [L, H, E], f32, kind="ExternalInput").ap()
    tau_d = nc.dram_tensor("tau", [L, S], f32, kind="ExternalInput").ap()
    wqT_d = nc.dram_tensor("wqT", [P, 2 * E], f32, kind="ExternalInput").ap()
    wkT_d = nc.dram_tensor("wkT", [P, 2 * E], f32, kind="ExternalInput").ap()
    wv2_d = nc.dram_tensor("wv2aug", [E + 1, E + 1], f32, kind="ExternalInput").ap()
    id_d = nc.dram_tensor("ident", [P, P], f32, kind="ExternalInput").ap()
    tri_d = nc.dram_tensor("tri", [P, P], f32, kind="ExternalInput").ap()
    sel_d = nc.dram_tensor("sel", [2, S, P], f32, kind="ExternalInput").ap()
    out_d = nc.dram_tensor("out", [L, H, E], f32, kind="ExternalOutput").ap()
    if dbg:
        dbg_d = {
            "trep01": nc.dram_tensor("dbg_trep01", [P, L], f32, kind="ExternalOutput").ap(),
            "trep23": nc.dram_tensor("dbg_trep23", [P, L], f32, kind="ExternalOutput").ap(),
            "xs_q": nc.dram_tensor("dbg_xs_q", [P, L + 1], f32, kind="ExternalOutput").ap(),
            "ctq0": nc.dram_tensor("dbg_ctq0", [P, L], f32, kind="ExternalOutput").ap(),
            "ctk0": nc.dram_tensor("dbg_ctk0", [P, L], f32, kind="ExternalOutput").ap(),
            "xi": nc.dram_tensor("dbg_xi", [P, NT, E + 1], f32, kind="ExternalOutput").ap(),
            "e00": nc.dram_tensor("dbg_e00", [P, 512], f32, kind="ExternalOutput").ap(),
            "e10": nc.dram_tensor("dbg_e10", [P, 512], f32, kind="ExternalOutput").ap(),
            "ots0": nc.dram_tensor("dbg_ots0", [E + 1, 512], f32, kind="ExternalOutput").ap(),
            "tq4": nc.dram_tensor("dbg_tq4", [P, NT, 1], f32, kind="ExternalOutput").ap(),
        }


    with tile.TileContext(nc) as tc:
        with ExitStack() as ctx:
            consts = ctx.enter_context(tc.tile_pool(name="consts", bufs=1))
            inp = ctx.enter_context(tc.tile_pool(name="inp", bufs=1))
            xt_ps = ctx.enter_context(tc.tile_pool(name="xt_ps", bufs=2, space="PSUM"))
            xt_sb = ctx.enter_context(tc.tile_pool(name="xt_sb", bufs=2))
            xd_ps = ctx.enter_context(tc.tile_pool(name="xd_ps", bufs=2, space="PSUM"))
            xsb = ctx.enter_context(tc.tile_pool(name="xsb", bufs=2))
            dpool = ctx.enter_context(tc.tile_pool(name="dpool", bufs=2))
            ctp = ctx.enter_context(tc.tile_pool(name="ctp", bufs=3))
            xip = ctx.enter_context(tc.tile_pool(name="xip", bufs=2))
            sc_ps = ctx.enter_context(tc.tile_pool(name="sc_ps", bufs=2, space="PSUM"))
            ep = ctx.enter_context(tc.tile_pool(name="ep", bufs=7))
            ot_ps = ctx.enter_context(tc.tile_pool(name="ot_ps", bufs=1, space="PSUM"))
            ot_sbp = ctx.enter_context(tc.tile_pool(name="ot_sbp", bufs=2))
            va_ps = ctx.enter_context(tc.tile_pool(name="va_ps", bufs=1, space="PSUM"))
            vop = ctx.enter_context(tc.tile_pool(name="vop", bufs=2))
            smallp = ctx.enter_context(tc.tile_pool(name="smallp", bufs=4))

            # ---- per-core constants ----
            ident = consts.tile([P, P], f32)
            nc.sync.dma_start(ident, id_d)
            tri = consts.tile([P, P], op_dt)
            tri32 = consts.tile([P, P], f32, tag="tri32")
            nc.sync.dma_start(tri32, tri_d)
            nc.vector.tensor_copy(tri, tri32)
            wqT = consts.tile([P, 2 * E], op_dt, tag="wqT")
            wkT = consts.tile([P, 2 * E], op_dt, tag="wkT")
            wq32 = consts.tile([P, 2 * E], f32, tag="wq32")
            wk32 = consts.tile([P, 2 * E], f32, tag="wk32")
            nc.sync.dma_start(wq32, wqT_d)
            nc.sync.dma_start(wk32, wkT_d)
            nc.vector.tensor_copy(wqT, wq32)
            nc.vector.tensor_copy(wkT, wk32)
            wv2 = consts.tile([E + 1, E + 1], f32)
            nc.sync.dma_start(wv2, wv2_d)

            # tau natural layout [p, t, s]; one efficient DMA.
            tau_nat = consts.tile([P, NT, S], f32)
            nc.sync.dma_start(
                tau_nat, tau_d.rearrange("(t p) s -> p t s", p=P)
            )
            tsum = consts.tile([P, NT, 1], f32)
            nc.vector.tensor_reduce(
                tsum, tau_nat, axis=mybir.AxisListType.X, op=Alu.add
            )
            tq4 = consts.tile([P, NT, 1], f32)
            nc.vector.tensor_scalar(tq4, tsum, 0.25, None, op0=Alu.mult)
            ones_e = consts.tile([P, E], f32, tag="ones_e")
            nc.vector.memset(ones_e, 1.0)
            if dbg:
                nc.sync.dma_start(dbg_d["tq4"], tq4)

            # Trep[p, l] = tau[l, 2c + p//64]: PE-transpose tau, then K=4
            # selector matmuls broadcast each tau column across 64 partitions.
            sel_sb = consts.tile([S, 2, P], f32, tag="sel")
            nc.sync.dma_start(sel_sb, sel_d.rearrange("c s p -> s c p"))
            tauT = consts.tile([S, L], f32, tag="tauT")
            for lc in range(2):
                tauT_ps = xt_ps.tile([S, 512], f32, tag="xtp")
                for t4 in range(4):
                    t = 4 * lc + t4
                    nc.tensor.transpose(
                        tauT_ps[:, t4 * P : (t4 + 1) * P], tau_nat[:, t, :], ident
                    )
                nc.scalar.copy(tauT[:, lc * 512 : (lc + 1) * 512], tauT_ps)
            treps = []
            for c in range(2):
                tr = consts.tile([P, L], el_dt, tag=f"trep{c}")
                for lc in range(2):
                    sl = slice(lc * 512, (lc + 1) * 512)
                    trep_ps = xd_ps.tile([P, 512], f32, tag="xdp")
                    nc.tensor.matmul(
                        trep_ps,
                        lhsT=sel_sb[:, c, :],
                        rhs=tauT[:, sl],
                        start=True,
                        stop=True,
                    )
                    nc.scalar.copy(tr[:, sl], trep_ps)
                treps.append(tr)
                if dbg:
                    nc.sync.dma_start(dbg_d["trep01" if c == 0 else "trep23"], tr)

            # Tq4 replicated along e for the one-shot xi multiply.
            tq4rep = consts.tile([P, NT, E], f32, tag="tq4rep")
            for t in range(NT):
                nc.vector.tensor_scalar(
                    tq4rep[:, t, :], ones_e, tq4[:, t, :], None, op0=Alu.mult
                )

            # ones column (in op_dt) for xi_aug; memset can't write f32r.
            ones32 = consts.tile([P, NT, 1], f32, tag="ones32")
            nc.vector.memset(ones32, 1.0)
            ones_c = consts.tile([P, NT, 1], op_dt, tag="ones_c")
            nc.vector.tensor_copy(ones_c, ones32)

            # one-shot whole-tensor loads (2 KiB descriptors); q and k are
            # interleaved per l-tile so one [128,128] PE transpose covers both.
            qk_all = inp.tile([P, NT, H, 2, E], f32, tag="qk_all")
            v_all = inp.tile([P, NT, H, E], f32, tag="v_all")
            qk_r = qk_d.rearrange("(t p) h x e -> p t h x e", p=P)
            for hh in range(H):
                nc.sync.dma_start(
                    qk_all[:, :, hh, :, :], qk_r[:, :, hh, :, :]
                )
            nc.sync.dma_start(
                v_all, v_d.rearrange("(t p) h e -> p t h e", p=P)
            )

            for h in range(H):
                qkx = qk_all[:, :, h, :, :]
                vx = v_all[:, :, h, :]
                vnx = vop.tile([P, NT, E], f32, tag="vnx")
                nc.sync.dma_start(
                    vnx[:, 0 : NT - 1, :],
                    v_d[1 : 1 + (NT - 1) * P, h, :].rearrange(
                        "(t p) e -> p t e", p=P
                    ),
                )
                nc.sync.dma_start(
                    vnx[0 : P - 1, NT - 1, :], v_d[(NT - 1) * P + 1 : L, h, :]
                )
                nc.sync.dma_start(vnx[P - 1 : P, NT - 1, :], v_d[L - 1 : L, h, :])

                # ---- transpose q+k together; project; build ct tensors ----
                # One [128,128] transpose per l-tile covers q (rows 0:64) and
                # k (rows 64:128); projections use zero-padded [128,128]
                # weights so both read the same combined transposed tile.
                xtqk = xt_sb.tile([P, L], op_dt, tag="xts")
                for lc in range(2):
                    xtp = xt_ps.tile([P, 512], f32, tag="xtp")
                    for t4 in range(4):
                        t = 4 * lc + t4
                        nc.tensor.transpose(
                            xtp[:, t4 * P : (t4 + 1) * P],
                            qkx[:, t, :, :],
                            ident,
                        )
                    nc.scalar.copy(xtqk[:, lc * 512 : (lc + 1) * 512], xtp)

                cts = {}
                xss = {}
                for name, wT in (("q", wqT), ("k", wkT)):
                    xs = xsb.tile([P, L + 1], el_dt, tag=f"xs_{name}")
                    xss[name] = xs
                    for lc in range(2):
                        sl = slice(lc * 512, (lc + 1) * 512)
                        xdp = xd_ps.tile([P, 512], f32, tag="xdp")
                        nc.tensor.matmul(
                            xdp, lhsT=wT, rhs=xtqk[:, sl], start=True, stop=True
                        )
                        nc.scalar.copy(xs[:, sl], xdp)
                        if lc == 1:
                            nc.vector.tensor_copy(
                                xs[:, L : L + 1], xdp[:, 511:512]
                            )

                    dd = dpool.tile([P, L], el_dt, tag=f"dd_{name}")
                    for lc in range(2):
                        sl = slice(lc * 512, (lc + 1) * 512)
                        sl1 = slice(lc * 512 + 1, (lc + 1) * 512 + 1)
                        nc.vector.tensor_tensor(
                            dd[:, sl], xs[:, sl1], xs[:, sl], op=Alu.subtract
                        )
                    for lc in range(2):
                        sl = slice(lc * 512, (lc + 1) * 512)
                        for c in range(2):
                            ct = ctp.tile([P, 512], op_dt, tag=f"ct_{name}{c}_{lc}")
                            cts[(name, c, lc)] = ct
                            nc.vector.tensor_tensor(
                                ct, dd[:, sl], treps[c][:, sl], op=Alu.mult
                            )
                            nc.gpsimd.tensor_tensor(
                                ct, ct, xs[:, sl], op=Alu.add
                            )

                    if dbg and h == 0 and name == "q":
                        nc.sync.dma_start(dbg_d["xs_q"], xs)

                # ---- xi (value-side interp, natural layout) + ones column ----
                xi = xip.tile([P, NT, E + 1], op_dt, tag="xi")
                dv = xip.tile([P, NT, E], f32, tag="dv")
                nc.vector.tensor_tensor(dv, vnx, vx, op=Alu.subtract)
                nc.vector.tensor_tensor(dv, dv, tq4rep, op=Alu.mult)
                nc.vector.tensor_tensor(xi[:, :, 0:E], dv, vx, op=Alu.add)
                nc.vector.tensor_copy(xi[:, :, E : E + 1], ones_c)
                if dbg and h == 0:
                    nc.sync.dma_start(dbg_d["xi"], xi.bitcast(f32))

                vo_all = vop.tile([P, NT, E], f32, tag="vo")

                # ---- scoresT -> exp (dense PE), then AV, per l-chunk ----
                for j in range(NJ):
                    otp = ot_ps.tile([E + 1, 512], f32, tag="otp")
                    ni = 4 * j + 4  # m-chunks 0..ni-1 participate
                    ets = []
                    for i in range(ni):
                        n0 = max(0, 128 * i - 512 * j)
                        sc = sc_ps.tile([P, 512], f32, tag="sc")
                        ilc, ioff = divmod(128 * i, 512)
                        for c in range(2):
                            nc.tensor.matmul(
                                sc[:, n0:512],
                                lhsT=cts[("k", c, ilc)][:, ioff : ioff + 128],
                                rhs=cts[("q", c, j)][:, n0:512],
                                start=(c == 0),
                                stop=(c == 1),
                            )
                        et = ep.tile([P, 512], op_dt, tag="et")
                        nc.scalar.activation(
                            et[:, n0:512], sc[:, n0:512], Exp, scale=float(EXP_SCALE)
                        )
                        if i >= 4 * j:  # diagonal block: triangular mask
                            nc.gpsimd.tensor_tensor(
                                et[:, n0 : n0 + 128],
                                et[:, n0 : n0 + 128],
                                tri,
                                op=Alu.mult,
                            )
                        ets.append((et, n0))
                        if dbg and h == 0 and j == 0 and i <= 1:
                            nc.sync.dma_start(dbg_d[f"e{i}0"], et.bitcast(f32))
                    for i, (et, n0) in enumerate(ets):
                        nc.tensor.matmul(
                            otp[:, n0:512],
                            lhsT=xi[:, i, :],
                            rhs=et[:, n0:512],
                            start=(i == 0),
                            stop=(i == ni - 1),
                        )
                    ots = ot_sbp.tile([E + 1, 512], f32, tag="ots")
                    nc.scalar.copy(ots, otp)
                    if dbg and h == 0 and j == 0:
                        nc.sync.dma_start(dbg_d["ots0"], ots)
                    vap = va_ps.tile([P, 4, E + 1], f32, tag="vap")
                    for q4 in range(4):
                        nc.tensor.matmul(
                            vap[:, q4, :],
                            lhsT=ots[:, q4 * 128 : (q4 + 1) * 128],
                            rhs=wv2,
                            start=True,
                            stop=True,
                        )
                    rec = smallp.tile([P, 4], f32, tag="rec")
                    nc.vector.reciprocal(rec, vap[:, :, E : E + 1])
                    for q4 in range(4):
                        nc.scalar.mul(
                            vo_all[:, 4 * j + q4, :],
                            vap[:, q4, 0:E],
                            rec[:, q4 : q4 + 1],
                        )

                nc.sync.dma_start(
                    out_d[:, h, :].rearrange("(t p) e -> p t e", p=P), vo_all
                )

    nc.compile()
    return nc


def _get_program(ct_bf16=False, dbg=False):
    key = ("prog", ct_bf16, dbg)
    if key not in _CACHE:
        _CACHE[key] = _build_program(ct_bf16, dbg)
    return _CACHE[key]


def _sel_const():
    sel = np.zeros((2, S, P), np.float32)
    for c in range(2):
        for p in range(P):
            sel[c, 2 * c + p // 64, p] = 1.0
    return sel


def _make_in_maps(inputs):
    """Per-core input maps: slice batch b for core b; replicate small consts."""
    queries = np.asarray(inputs["queries"], dtype=np.float32)
    keys = np.asarray(inputs["keys"], dtype=np.float32)
    values = np.asarray(inputs["values"], dtype=np.float32)
    his = np.asarray(inputs["his_timeslot"], dtype=np.float32)
    Wq = np.asarray(inputs["Wq"], dtype=np.float32)
    Wk = np.asarray(inputs["Wk"], dtype=np.float32)
    Wv = np.asarray(inputs["Wv"], dtype=np.float32)

    ident = np.eye(P, dtype=np.float32)
    tri = np.triu(np.ones((P, P), dtype=np.float32))
    sel = _sel_const()
    wqT = np.zeros((P, 2 * E), np.float32)
    wqT[0:E] = np.concatenate([Wq.T, Wq.T], axis=1)
    wkT = np.zeros((P, 2 * E), np.float32)
    wkT[E : 2 * E] = np.concatenate([Wk.T, Wk.T], axis=1)
    wv2 = np.zeros((E + 1, E + 1), dtype=np.float32)
    wv2[:E, :E] = 2.0 * Wv.T
    wv2[E, E] = 1.0

    in_maps = []
    for b in range(B):
        in_maps.append(
            {
                "qk": np.ascontiguousarray(
                    np.stack([queries[b], keys[b]], axis=2)
                ),
                "v": np.ascontiguousarray(values[b]),
                "tau": np.ascontiguousarray(his[b]),
                "wqT": wqT,
                "wkT": wkT,
                "wv2aug": wv2,
                "ident": ident,
                "tri": tri,
                "sel": sel,
            }
        )
    return in_maps


def kernel(queries, keys, values, his_timeslot, label_pre_timeslot, attn_mask,
           Wq, bq, Wk, bk, Wv, bv):
    from concourse import bass_utils

    bq = np.asarray(bq, dtype=np.float32)
    bk = np.asarray(bk, dtype=np.float32)
    bv = np.asarray(bv, dtype=np.float32)
    assert np.all(bq == 0) and np.all(bk == 0), (
        "kernel specialized for zero q/k biases (as produced by setup_inputs)"
    )

    nc = _get_program(ct_bf16=False)
    in_maps = _make_in_maps(
        {
            "queries": queries,
            "keys": keys,
            "values": values,
            "his_timeslot": his_timeslot,
            "Wq": Wq,
            "Wk": Wk,
            "Wv": Wv,
        }
    )
    res = bass_utils.run_bass_kernel_spmd(nc, in_maps, core_ids=list(range(B)))
    out = np.stack([res.results[b]["out"] for b in range(B)], axis=0)
    if np.any(bv != 0):
        # rows of the softmax sum to 1, so the value bias contributes
        # exactly 2*bv to every output position (handled host-side, exact).
        out = out + 2.0 * bv[None, None, None, :]
    return out.astype(np.float32)



# revision 13
# speedup vs baseline: 1.8011x; 1.8011x over previous
"""Trainium2 Bass kernel for nn_CTAttention (continuous-time sparse attention).

Shapes (hardcoded): B=8, L=1024, H=8, E=64, S=4.
Sharding: data-parallel over B (one batch element per NeuronCore, 8 cores),
head loop inside each core; the small E x E weights are replicated.

Math (per b, h), with tau = his_timeslot[b] (shared by q/k/v interp):
  Xq[f, l]   = sum_e Wq[f, e] x[l, e]          (projection commutes with the
                                                linear time-interp, so project
                                                first, interp after)
  ct_q[(s,f), l] = Xq[f, l] + tau[l, s] * (Xq[f, l+1] - Xq[f, l])   (clamped)
  scoresT[m, l]  = sum_{s,f} ct_k[(s,f), m] ct_q[(s,f), l]
  E = exp(0.0625 * scoresT); causal masking is done by accumulating
      stepT^T @ (-32768*I) into the diagonal score blocks on the PE (adds
      -32768 where l < m, so exp underflows to +0.0) -- no elementwise mask.
  xi[m, :] = v[m] + (sum_s tau[m,s]/4) * (v[m+1] - v[m]);  v_bar = 2*Wv@xi
  OT[e', l] = sum_m xi_aug[m, e'] E[m, l]   (xi_aug has a ones column ->
                                             row 64 of OT = softmax denom)
  V[l, f] = (sum_e OT[e, l] * 2Wv^T[e, f]) / denom[l]
Biases bq/bk are zero in this problem (asserted); bv is handled exactly by
adding 2*bv to the output on the host (rows of softmax sum to 1).

All matmul operands and elementwise intermediates are bf16 (PSUM accumulation
stays f32); measured end-to-end rel err ~1e-2 vs the f32 reference, inside
the 2e-2 gate. Elementwise work is split: DVE (ct build, dd, xi, output
scaling), Act (exp, PSUM evictions); the Pool engine is kept off the hot
path because its SBUF port contends with the DVE. The head loop is a
depth-2 software pipeline: head h+2's transpose/project/interp chain is
emitted before head h's score/AV mainloop so the PE stays dense (HAM warm).
"""

import numpy as np

B, L, H, E, S = 8, 1024, 8, 64, 4
P = 128           # partitions
NT = L // P       # 8 l-tiles of 128
NJ = L // 512     # 2 l-chunks of 512
EXP_SCALE = 0.5 / np.sqrt(E)  # 0.5 * SCALE = 0.5/8 = 0.0625

_CACHE = {}


def _build_program(ct_bf16=True):
    from contextlib import ExitStack

    import concourse.bass as bass
    import concourse.tile as tile
    from concourse import bacc, mybir

    f32 = mybir.dt.float32
    f32r = mybir.dt.float32r
    bf16 = mybir.dt.bfloat16
    Exp = mybir.ActivationFunctionType.Exp
    Alu = mybir.AluOpType

    nc = bacc.Bacc("TRN2", debug=False, enable_asserts=False, num_devices=8)

    qk_d = nc.dram_tensor("qk", [L, H, 2, E], f32, kind="ExternalInput").ap()
    v_d = nc.dram_tensor("v", [L, H, E], f32, kind="ExternalInput").ap()
    tau_d = nc.dram_tensor("tau", [L, S], f32, kind="ExternalInput").ap()
    wqT_d = nc.dram_tensor("wqT", [P, 2 * E], f32, kind="ExternalInput").ap()
    wkT_d = nc.dram_tensor("wkT", [P, 2 * E], f32, kind="ExternalInput").ap()
    wv2_d = nc.dram_tensor("wv2aug", [E + 1, E + 1], f32, kind="ExternalInput").ap()
    id_d = nc.dram_tensor("ident", [P, P], f32, kind="ExternalInput").ap()
    step_d = nc.dram_tensor("stepT", [P, P], f32, kind="ExternalInput").ap()
    negi_d = nc.dram_tensor("negI", [P, P], f32, kind="ExternalInput").ap()
    sel_d = nc.dram_tensor("sel", [2, S, P], f32, kind="ExternalInput").ap()
    out_d = nc.dram_tensor("out", [L, H, E], f32, kind="ExternalOutput").ap()

    with tile.TileContext(nc) as tc:
        with ExitStack() as ctx:
            consts = ctx.enter_context(tc.tile_pool(name="consts", bufs=1))
            inp = ctx.enter_context(tc.tile_pool(name="inp", bufs=1))
            xt_ps = ctx.enter_context(tc.tile_pool(name="xt_ps", bufs=2, space="PSUM"))
            xt_sb = ctx.enter_context(tc.tile_pool(name="xt_sb", bufs=3))
            xd_ps = ctx.enter_context(tc.tile_pool(name="xd_ps", bufs=2, space="PSUM"))
            xsb = ctx.enter_context(tc.tile_pool(name="xsb", bufs=3))
            dpool = ctx.enter_context(tc.tile_pool(name="dpool", bufs=2))
            ctp = ctx.enter_context(tc.tile_pool(name="ctp", bufs=3))
            xip = ctx.enter_context(tc.tile_pool(name="xip", bufs=3))
            sc_ps = ctx.enter_context(tc.tile_pool(name="sc_ps", bufs=2, space="PSUM"))
            ep = ctx.enter_context(tc.tile_pool(name="ep", bufs=6))
            ot_ps = ctx.enter_context(tc.tile_pool(name="ot_ps", bufs=1, space="PSUM"))
            ot_sbp = ctx.enter_context(tc.tile_pool(name="ot_sbp", bufs=2))
            va_ps = ctx.enter_context(tc.tile_pool(name="va_ps", bufs=1, space="PSUM"))
            vop = ctx.enter_context(tc.tile_pool(name="vop", bufs=3))
            smallp = ctx.enter_context(tc.tile_pool(name="smallp", bufs=4))

            # ---- per-core constants ----
            ident = consts.tile([P, P], f32)
            nc.sync.dma_start(ident, id_d)

            stepT = consts.tile([P, P], bf16, tag="stepT")
            negI = consts.tile([P, P], bf16, tag="negI")
            s32 = consts.tile([P, P], f32, tag="s32")
            n32 = consts.tile([P, P], f32, tag="n32")
            nc.sync.dma_start(s32, step_d)
            nc.sync.dma_start(n32, negi_d)
            nc.vector.tensor_copy(stepT, s32)
            nc.vector.tensor_copy(negI, n32)

            wqT = consts.tile([P, 2 * E], bf16, tag="wqT")
            wkT = consts.tile([P, 2 * E], bf16, tag="wkT")
            wq32 = consts.tile([P, 2 * E], f32, tag="wq32")
            wk32 = consts.tile([P, 2 * E], f32, tag="wk32")
            nc.sync.dma_start(wq32, wqT_d)
            nc.sync.dma_start(wk32, wkT_d)
            nc.vector.tensor_copy(wqT, wq32)
            nc.vector.tensor_copy(wkT, wk32)

            wv2 = consts.tile([E + 1, E + 1], bf16, tag="wv2")
            wv32 = consts.tile([E + 1, E + 1], f32, tag="wv32")
            nc.sync.dma_start(wv32, wv2_d)
            nc.vector.tensor_copy(wv2, wv32)

            # tau natural layout [p, t, s]; one efficient DMA.
            tau_nat = consts.tile([P, NT, S], f32)
            nc.sync.dma_start(
                tau_nat, tau_d.rearrange("(t p) s -> p t s", p=P)
            )
            tsum = consts.tile([P, NT, 1], f32)
            nc.vector.tensor_reduce(
                tsum, tau_nat, axis=mybir.AxisListType.X, op=Alu.add
            )
            tq4 = consts.tile([P, NT, 1], f32)
            nc.vector.tensor_scalar(tq4, tsum, 0.25, None, op0=Alu.mult)
            ones_e = consts.tile([P, E], f32, tag="ones_e")
            nc.vector.memset(ones_e, 1.0)

            # Trep[p, l] = tau[l, 2c + p//64]: PE-transpose tau, then K=4
            # selector matmuls broadcast each tau column across 64 partitions.
            sel32 = consts.tile([S, 2, P], f32, tag="sel32")
            sel_sb = consts.tile([S, 2, P], bf16, tag="sel")
            nc.sync.dma_start(sel32, sel_d.rearrange("c s p -> s c p"))
            nc.vector.tensor_copy(sel_sb, sel32)
            tauT = consts.tile([S, L], bf16, tag="tauT")
            for lc in range(2):
                tauT_ps = xt_ps.tile([S, 512], f32, tag="xtp")
                for t4 in range(4):
                    t = 4 * lc + t4
                    nc.tensor.transpose(
                        tauT_ps[:, t4 * P : (t4 + 1) * P], tau_nat[:, t, :], ident
                    )
                nc.scalar.copy(tauT[:, lc * 512 : (lc + 1) * 512], tauT_ps)
            treps = []
            for c in range(2):
                tr = consts.tile([P, L], bf16, tag=f"trep{c}")
                for lc in range(2):
                    sl = slice(lc * 512, (lc + 1) * 512)
                    trep_ps = xd_ps.tile([P, 512], f32, tag="xdp")
                    nc.tensor.matmul(
                        trep_ps,
                        lhsT=sel_sb[:, c, :],
                        rhs=tauT[:, sl],
                        start=True,
                        stop=True,
                    )
                    nc.scalar.copy(tr[:, sl], trep_ps)
                treps.append(tr)

            # Tq4 replicated along e for the one-shot xi multiply.
            tq4rep = consts.tile([P, NT, E], f32, tag="tq4rep")
            for t in range(NT):
                nc.vector.tensor_scalar(
                    tq4rep[:, t, :], ones_e, tq4[:, t, :], None, op0=Alu.mult
                )

            # ones column (bf16) for xi_aug; memset can't write bf16.
            ones32 = consts.tile([P, NT, 1], f32, tag="ones32")
            nc.vector.memset(ones32, 1.0)
            ones_c = consts.tile([P, NT, 1], bf16, tag="ones_c")
            nc.vector.tensor_copy(ones_c, ones32)

            # one-shot whole-tensor loads (2 KiB descriptors); q and k are
            # interleaved per l-tile so one [128,128] PE transpose covers both.
            qk_all = inp.tile([P, NT, H, 2, E], f32, tag="qk_all")
            qk_r = qk_d.rearrange("(t p) h x e -> p t h x e", p=P)
            v_all = inp.tile([P, NT, H, E], f32, tag="v_all")
            v_r = v_d.rearrange("(t p) h e -> p t h e", p=P)

            # persistent per-parity tiles: xi keeps its ones column across
            # heads; dd keeps its zero last column (clamp: x_{L}==x_{L-1}).
            xis, dds = [], []
            for par in range(4):
                xi_t = consts.tile([P, NT, E + 1], bf16, tag=f"xi{par}")
                nc.vector.tensor_copy(xi_t[:, :, E : E + 1], ones_c)
                xis.append(xi_t)
            for par in range(2):
                dq = consts.tile([P, L], bf16, tag=f"ddq{par}")
                dk = consts.tile([P, L], bf16, tag=f"ddk{par}")
                nc.vector.memset(dq[:, L - 2 : L].bitcast(f32), 0.0)
                nc.vector.memset(dk[:, L - 2 : L].bitcast(f32), 0.0)
                dds.append({"q": dq, "k": dk})

            state = {}

            def pre(h):
                """transpose+project+interp build for head h (PE+DVE+Pool)."""
                par = h % 2
                xpar = h % 4
                qkx = qk_all[:, :, h, :, :]
                vx = v_all[:, :, h, :]
                nc.sync.dma_start(qkx, qk_r[:, :, h, :, :])
                nc.sync.dma_start(vx, v_r[:, :, h, :])
                vnx = vop.tile([P, NT, E], f32, tag="vnx")
                nc.sync.dma_start(
                    vnx[:, 0 : NT - 1, :],
                    v_d[1 : 1 + (NT - 1) * P, h, :].rearrange(
                        "(t p) e -> p t e", p=P
                    ),
                )
                nc.sync.dma_start(
                    vnx[0 : P - 1, NT - 1, :], v_d[(NT - 1) * P + 1 : L, h, :]
                )
                nc.sync.dma_start(vnx[P - 1 : P, NT - 1, :], v_d[L - 1 : L, h, :])

                xtqk = xt_sb.tile([P, L], bf16, tag="xts")
                for lc in range(2):
                    xtp = xt_ps.tile([P, 512], f32, tag="xtp")
                    for t4 in range(4):
                        t = 4 * lc + t4
                        nc.tensor.transpose(
                            xtp[:, t4 * P : (t4 + 1) * P],
                            qkx[:, t, :, :],
                            ident,
                        )
                    nc.scalar.copy(xtqk[:, lc * 512 : (lc + 1) * 512], xtp)

                cts = {}
                for name, wT in (("q", wqT), ("k", wkT)):
                    xs = xsb.tile([P, L], bf16, tag=f"xs_{name}")
                    for lc in range(2):
                        sl = slice(lc * 512, (lc + 1) * 512)
                        xdp = xd_ps.tile([P, 512], f32, tag="xdp")
                        nc.tensor.matmul(
                            xdp, lhsT=wT, rhs=xtqk[:, sl], start=True, stop=True
                        )
                        nc.scalar.copy(xs[:, sl], xdp)

                    # dd[l] = xs[l+1] - xs[l] for l<L-1; dd[L-1]=0 (persistent)
                    dd = dds[par][name]
                    nc.vector.tensor_tensor(
                        dd[:, 0 : L - 1], xs[:, 1:L], xs[:, 0 : L - 1],
                        op=Alu.subtract,
                    )
                    for c in range(2):
                        ct = ctp.tile([P, L], bf16, tag=f"ct_{name}{c}")
                        cts[(name, c)] = ct
                        nc.vector.tensor_tensor(ct, dd, treps[c], op=Alu.mult)
                        nc.vector.tensor_tensor(ct, ct, xs, op=Alu.add)

                # xi (value-side interp, natural layout); ones col persistent
                xi = xis[xpar]
                dv = xip.tile([P, NT, E], f32, tag="dv")
                nc.vector.tensor_tensor(dv, vnx, vx, op=Alu.subtract)
                nc.vector.tensor_tensor(dv, dv, tq4rep, op=Alu.mult)
                nc.vector.tensor_tensor(xi[:, :, 0:E], dv, vx, op=Alu.add)
                state[h] = cts

            def main(h):
                """scoresT -> exp -> AV -> project-back for head h."""
                cts = state.pop(h)
                xi = xis[h % 4]
                vo_all = vop.tile([P, NT, E], f32, tag="vo")
                for j in range(NJ):
                    otp = ot_ps.tile([E + 1, 512], f32, tag="otp")
                    ni = 4 * j + 4  # m-chunks 0..ni-1 participate
                    ets = []
                    for i in range(ni):
                        n0 = max(0, 128 * i - 512 * j)
                        sc = sc_ps.tile([P, 512], f32, tag="sc")
                        diag = i >= 4 * j
                        for c in range(2):
                            nc.tensor.matmul(
                                sc[:, n0:512],
                                lhsT=cts[("k", c)][:, 128 * i : 128 * i + 128],
                                rhs=cts[("q", c)][:, 512 * j + n0 : 512 * (j + 1)],
                                start=(c == 0),
                                stop=(c == 1 and not diag),
                            )
                        if diag:
                            # causal mask: adds -32768 where l < m within the
                            # diagonal 128-block (exp underflows to +0.0)
                            nc.tensor.matmul(
                                sc[:, n0 : n0 + 128],
                                lhsT=stepT,
                                rhs=negI,
                                start=False,
                                stop=True,
                            )
                        et = ep.tile([P, 512], bf16, tag="et")
                        nc.scalar.activation(
                            et[:, n0:512], sc[:, n0:512], Exp, scale=float(EXP_SCALE)
                        )
                        ets.append((et, n0))
                    for i, (et, n0) in enumerate(ets):
                        nc.tensor.matmul(
                            otp[:, n0:512],
                            lhsT=xi[:, i, :],
                            rhs=et[:, n0:512],
                            start=(i == 0),
                            stop=(i == ni - 1),
                        )
                    ots = ot_sbp.tile([E + 1, 512], bf16, tag="ots")
                    nc.scalar.copy(ots, otp)
                    vap = va_ps.tile([P, 4, E + 1], f32, tag="vap")
                    for q4 in range(4):
                        nc.tensor.matmul(
                            vap[:, q4, :],
                            lhsT=ots[:, q4 * 128 : (q4 + 1) * 128],
                            rhs=wv2,
                            start=True,
                            stop=True,
                        )
                    rec = smallp.tile([P, 4], f32, tag="rec")
                    nc.vector.reciprocal(rec, vap[:, :, E : E + 1])
                    for q4 in range(4):
                        nc.vector.tensor_scalar(
                            vo_all[:, 4 * j + q4, :],
                            vap[:, q4, 0:E],
                            rec[:, q4 : q4 + 1],
                            None,
                            op0=Alu.mult,
                        )

                nc.sync.dma_start(
                    out_d[:, h, :].rearrange("(t p) e -> p t e", p=P), vo_all
                )

            # software pipeline: preamble of head h+1 is emitted (and runs)
            # during the score/AV mainloop of head h, keeping the PE dense.
            pre(0)
            pre(1)
            for h in range(H):
                if h + 2 < H:
                    pre(h + 2)
                main(h)

    nc.compile()
    return nc


def _get_program(ct_bf16=True):
    # the kernel is always bf16 now; the arg is kept for test.py compat
    key = "prog"
    if key not in _CACHE:
        _CACHE[key] = _build_program()
    return _CACHE[key]


def _sel_const():
    sel = np.zeros((2, S, P), np.float32)
    for c in range(2):
        for p in range(P):
            sel[c, 2 * c + p // 64, p] = 1.0
    return sel


def _make_in_maps(inputs):
    """Per-core input maps: slice batch b for core b; replicate small consts."""
    queries = np.asarray(inputs["queries"], dtype=np.float32)
    keys = np.asarray(inputs["keys"], dtype=np.float32)
    values = np.asarray(inputs["values"], dtype=np.float32)
    his = np.asarray(inputs["his_timeslot"], dtype=np.float32)
    Wq = np.asarray(inputs["Wq"], dtype=np.float32)
    Wk = np.asarray(inputs["Wk"], dtype=np.float32)
    Wv = np.asarray(inputs["Wv"], dtype=np.float32)

    ident = np.eye(P, dtype=np.float32)
    stepT = np.triu(np.ones((P, P), dtype=np.float32), 1)  # [p, m] = 1 if p < m
    negI = -32768.0 * np.eye(P, dtype=np.float32)
    sel = _sel_const()
    wqT = np.zeros((P, 2 * E), np.float32)
    wqT[0:E] = np.concatenate([Wq.T, Wq.T], axis=1)
    wkT = np.zeros((P, 2 * E), np.float32)
    wkT[E : 2 * E] = np.concatenate([Wk.T, Wk.T], axis=1)
    wv2 = np.zeros((E + 1, E + 1), dtype=np.float32)
    wv2[:E, :E] = 2.0 * Wv.T
    wv2[E, E] = 1.0

    in_maps = []
    for b in range(B):
        in_maps.append(
            {
                "qk": np.ascontiguousarray(
                    np.stack([queries[b], keys[b]], axis=2)
                ),
                "v": np.ascontiguousarray(values[b]),
                "tau": np.ascontiguousarray(his[b]),
                "wqT": wqT,
                "wkT": wkT,
                "wv2aug": wv2,
                "ident": ident,
                "stepT": stepT,
                "negI": negI,
                "sel": sel,
            }
        )
    return in_maps


def kernel(queries, keys, values, his_timeslot, label_pre_timeslot, attn_mask,
           Wq, bq, Wk, bk, Wv, bv):
    from concourse import bass_utils

    bq = np.asarray(bq, dtype=np.float32)
    bk = np.asarray(bk, dtype=np.float32)
    bv = np.asarray(bv, dtype=np.float32)
    assert np.all(bq == 0) and np.all(bk == 0), (
        "kernel specialized for zero q/k biases (as produced by setup_inputs)"
    )

    nc = _get_program()
    in_maps = _make_in_maps(
        {
            "queries": queries,
            "keys": keys,
            "values": values,
            "his_timeslot": his_timeslot,
            "Wq": Wq,
            "Wk": Wk,
            "Wv": Wv,
        }
    )
    res = bass_utils.run_bass_kernel_spmd(nc, in_maps, core_ids=list(range(B)))
    out = np.stack([res.results[b]["out"] for b in range(B)], axis=0)
    if np.any(bv != 0):
        # rows of the softmax sum to 1, so the value bias contributes
        # exactly 2*bv to every output position (handled host-side, exact).
        out = out + 2.0 * bv[None, None, None, :]
    return out.astype(np.float32)
